# revision 7
# baseline (speedup 1.0000x reference)
"""Trainium2 Bass kernel for MQA attention with RMSNorm + positional bias.

Reference computation (per core, seq-sharded over 8 cores):
  xn = rmsnorm(x) * gamma
  q = (xn @ wq) * scale   (16 heads x 128)     k = xn @ wk    v = xn @ wv
  sim = q @ k^T + pos_bias ; masked (non-causal entries := 1e-10)
  attn = softmax(sim); out = (attn @ v, concat heads) @ wo

Sharding: core m owns query rows [256*m, 256*m+256). K/V (shared MQA head)
are computed replicated on every core from the full x. Each core emits its
256 rows of the final output; the host concatenates. No collectives.

Precision: q/k projections and q@k^T run in true fp32 (the softmax here is
argmax-sharp: logits have std ~2000, so low-precision matmuls flip argmax
rows and blow up the error). v projection, attn@v and the output projection
run in bf16 - these only need ~1e-3 relative accuracy.

Attention inner loop is software-pipelined: the PE stream for head h's
sim matmuls is emitted before head h-1's P^T transposes + attn@v, so the
PE works on h-1's tail while DVE/ACT run h's softmax.
"""

import os

import numpy as np

import concourse.bass as bass
import concourse.mybir as mybir
import concourse.tile as tile
from concourse import bacc, masks
from concourse.bass_utils import run_bass_kernel_spmd

SEQ = 2048
DIM = 2048
H = 16
DH = 128
P = 128
N_CORES = 8
MQ = SEQ // N_CORES      # 256 query rows per core
NQT = MQ // P            # 2 query tiles per core
CD = DIM // P            # 16 contraction chunks
NS = SEQ // P            # 16 seq tiles
SPG = 2                  # seq tiles per k/v projection group
SG = NS // SPG           # 8 groups
SCALE = DH ** -0.5
EPS = 1e-5
MASKV = 1e-10

FP = mybir.dt.float32
BF = mybir.dt.bfloat16
U8 = mybir.dt.uint8
AF = mybir.ActivationFunctionType
ALU = mybir.AluOpType
AX = mybir.AxisListType

last_exec_time_ns = None


def _rms_scale_rows(nc, pool, xt, tag):
    """In-place x *= rsqrt(mean(x^2)+eps) for a [P, DIM] tile."""
    sq = pool.tile([P, DIM], FP, tag="sq_scratch", name="sq_scratch", bufs=1)
    ssq = pool.tile([P, 1], FP, tag=f"ssq{tag}", name=f"ssq{tag}")
    nc.scalar.activation(sq[:], xt[:], AF.Square, accum_out=ssq[:])
    nc.vector.tensor_scalar(ssq[:], ssq[:], 1.0 / DIM, EPS, ALU.mult, ALU.add)
    nc.scalar.sqrt(ssq[:], ssq[:])
    nc.vector.reciprocal(ssq[:], ssq[:])
    nc.vector.tensor_scalar_mul(xt[:], xt[:], ssq[:])


def build():
    nc = bacc.Bacc("TRN2", target_bir_lowering=False, debug=False,
                   num_devices=N_CORES)
    x_d = nc.dram_tensor("x", [SEQ, DIM], FP, kind="ExternalInput")
    xq_d = nc.dram_tensor("xq", [MQ, DIM], FP, kind="ExternalInput")
    pb_d = nc.dram_tensor("pb", [H * MQ, SEQ], FP, kind="ExternalInput")
    minv_d = nc.dram_tensor("minv", [MQ, SEQ], U8, kind="ExternalInput")
    g_d = nc.dram_tensor("gamma_t", [P, CD], FP, kind="ExternalInput")
    wq_d = nc.dram_tensor("wq", [DIM, H * DH], FP, kind="ExternalInput")
    wk_d = nc.dram_tensor("wk", [DIM, DH], FP, kind="ExternalInput")
    wv_d = nc.dram_tensor("wv", [DIM, DH], FP, kind="ExternalInput")
    wo_d = nc.dram_tensor("wo", [H * DH, DIM], FP, kind="ExternalInput")
    out_d = nc.dram_tensor("out", [MQ, DIM], FP, kind="ExternalOutput")

    with tile.TileContext(nc) as tc, \
         tc.tile_pool(name="singles", bufs=1) as singles:
        # ---- persistent tiles --------------------------------------------
        ident = singles.tile([P, P], FP, tag="ident", name="ident")
        masks.make_identity(nc, ident[:])
        identb = singles.tile([P, P], BF, tag="identb", name="identb")
        masks.make_identity(nc, identb[:])
        gam = singles.tile([P, CD], FP, tag="gam", name="gam")
        nc.sync.dma_start(out=gam[:], in_=g_d[:])
        minv = singles.tile([P, NQT, SEQ], U8, tag="minv", name="minv")
        for t in range(NQT):
            nc.sync.dma_start(out=minv[:, t, :], in_=minv_d[t * P:(t + 1) * P, :])
        cfill = singles.tile([P, SEQ], FP, tag="cfill", name="cfill")
        nc.gpsimd.memset(cfill[:], MASKV)

        qT = singles.tile([P, H, MQ], FP, tag="qT", name="qT")
        kT = singles.tile([P, SEQ], FP, tag="kT", name="kT")
        vsb = singles.tile([P, NS, DH], BF, tag="vsb", name="vsb")
        oT = singles.tile([P, H, MQ], BF, tag="oT", name="oT")

        with tc.tile_pool(name="xnTqp", bufs=1) as xnTqp:
            xnTq = xnTqp.tile([P, CD, MQ], FP, tag="xnTq", name="xnTq")

            # ---- phase 0: own-row xn^T -----------------------------------
            with tc.tile_pool(name="ph0", bufs=2) as ph0, \
                 tc.tile_pool(name="pstr0", bufs=2, space="PSUM") as pstr0:
                xnq = []
                for t in range(NQT):
                    xt = ph0.tile([P, DIM], FP, tag=f"xq{t}", name=f"xq{t}")
                    nc.sync.dma_start(out=xt[:], in_=xq_d[t * P:(t + 1) * P, :])
                    _rms_scale_rows(nc, ph0, xt, f"q{t}")
                    xnq.append(xt)
                for c in range(CD):
                    pt = pstr0.tile([P, MQ], FP, tag="trq", name="trq")
                    for t in range(NQT):
                        nc.tensor.transpose(pt[:, t * P:(t + 1) * P],
                                            xnq[t][:, c * P:(c + 1) * P],
                                            ident[:])
                    nc.vector.tensor_scalar_mul(xnTq[:, c, :], pt[:],
                                                gam[:, c:c + 1])

            # ---- phase 2: k/v projection over full seq -------------------
            with tc.tile_pool(name="xs", bufs=2) as xsp, \
                 tc.tile_pool(name="kvw", bufs=1) as kvwp, \
                 tc.tile_pool(name="pstr", bufs=2, space="PSUM") as pstr, \
                 tc.tile_pool(name="psk", bufs=2, space="PSUM") as psk, \
                 tc.tile_pool(name="psv", bufs=2, space="PSUM") as psv:
                wk_sb = kvwp.tile([P, CD, DH], FP, tag="wk", name="wk_sb")
                wv_sb = kvwp.tile([P, CD, DH], FP, tag="wv", name="wv_sb")
                wv_bf = kvwp.tile([P, CD, DH], BF, tag="wvb", name="wv_bf")
                for c in range(CD):
                    nc.sync.dma_start(out=wk_sb[:, c, :],
                                      in_=wk_d[c * P:(c + 1) * P, :])
                    nc.sync.dma_start(out=wv_sb[:, c, :],
                                      in_=wv_d[c * P:(c + 1) * P, :])
                    nc.scalar.copy(wv_bf[:, c, :], wv_sb[:, c, :])
                xnT = kvwp.tile([P, CD, SPG * P], FP, tag="xnT", name="xnT")
                xnTb = kvwp.tile([P, CD, SPG * P], BF, tag="xnTb", name="xnTb")
                vTs = kvwp.tile([P, SPG * P], FP, tag="vTs", name="vTs")
                for sg in range(SG):
                    xns = []
                    for s4 in range(SPG):
                        s = sg * SPG + s4
                        xt = xsp.tile([P, DIM], FP, tag=f"xs{s4}",
                                      name=f"xs{s4}")
                        nc.sync.dma_start(out=xt[:],
                                          in_=x_d[s * P:(s + 1) * P, :])
                        _rms_scale_rows(nc, xsp, xt, f"s{s4}")
                        xns.append(xt)
                    for c in range(CD):
                        pt = pstr.tile([P, SPG * P], FP, tag="trs", name="trs")
                        for s4 in range(SPG):
                            nc.tensor.transpose(pt[:, s4 * P:(s4 + 1) * P],
                                                xns[s4][:, c * P:(c + 1) * P],
                                                ident[:])
                        nc.vector.tensor_scalar_mul(xnT[:, c, :], pt[:],
                                                    gam[:, c:c + 1])
                        nc.scalar.copy(xnTb[:, c, :], xnT[:, c, :])
                    pk = psk.tile([P, SPG * P], FP, tag="pk", name="pk")
                    for c in range(CD):
                        nc.tensor.matmul(pk[:], lhsT=wk_sb[:, c, :],
                                         rhs=xnT[:, c, :],
                                         start=(c == 0), stop=(c == CD - 1))
                    nc.scalar.copy(kT[:, sg * SPG * P:(sg + 1) * SPG * P],
                                   pk[:])
                    pv = psv.tile([P, SPG * P], FP, tag="pv", name="pv")
                    for c in range(CD):
                        nc.tensor.matmul(pv[:], lhsT=wv_bf[:, c, :],
                                         rhs=xnTb[:, c, :],
                                         start=(c == 0), stop=(c == CD - 1))
                    nc.vector.tensor_copy(vTs[:], pv[:])
                    for s4 in range(SPG):
                        s = sg * SPG + s4
                        ptv = pstr.tile([P, P], FP, tag="vtr", name="vtr")
                        nc.tensor.transpose(ptv[:],
                                            vTs[:, s4 * P:(s4 + 1) * P],
                                            ident[:])
                        nc.vector.tensor_copy(vsb[:, s, :], ptv[:])

            # ---- phase 1: q projection (fp32) ----------------------------
            with tc.tile_pool(name="wqp", bufs=8) as wqp, \
                 tc.tile_pool(name="psq", bufs=2, space="PSUM") as psq:
                for h in range(H):
                    pq = psq.tile([P, MQ], FP, tag="pq", name="pq")
                    for c in range(CD):
                        wt = wqp.tile([P, P], FP, tag="wq", name="wqt")
                        nc.sync.dma_start(
                            out=wt[:],
                            in_=wq_d[c * P:(c + 1) * P, h * DH:(h + 1) * DH])
                        nc.tensor.matmul(pq[:], lhsT=wt[:], rhs=xnTq[:, c, :],
                                         start=(c == 0), stop=(c == CD - 1))
                    nc.scalar.mul(qT[:, h, :], pq[:], SCALE)

        # ---- phase 3: attention, software-pipelined over heads -----------
        with tc.tile_pool(name="pos", bufs=3) as posp, \
             tc.tile_pool(name="simp", bufs=2) as simp, \
             tc.tile_pool(name="pp", bufs=4) as ppool, \
             tc.tile_pool(name="pts", bufs=2) as ptsp, \
             tc.tile_pool(name="st", bufs=8) as stp, \
             tc.tile_pool(name="ps_sim", bufs=4, space="PSUM") as ps_sim, \
             tc.tile_pool(name="ps_pt", bufs=2, space="PSUM") as ps_pt, \
             tc.tile_pool(name="ps_o", bufs=2, space="PSUM") as ps_o:

            def sim_softmax(h):
                """Emit sim matmuls + softmax for head h; return pexp tiles."""
                pexps = []
                for t in range(NQT):
                    pos_t = posp.tile([P, SEQ], FP, tag="pos", name="pos")
                    nc.sync.dma_start(
                        out=pos_t[:],
                        in_=pb_d[h * MQ + t * P: h * MQ + (t + 1) * P, :])
                    sim = simp.tile([P, SEQ], FP, tag="sim", name="sim")
                    for nk in range(SEQ // 512):
                        psim = ps_sim.tile([P, 512], FP, tag="psim",
                                           name="psim")
                        nc.tensor.matmul(psim[:],
                                         lhsT=qT[:, h, t * P:(t + 1) * P],
                                         rhs=kT[:, nk * 512:(nk + 1) * 512],
                                         start=True, stop=True)
                        nc.vector.tensor_tensor(
                            sim[:, nk * 512:(nk + 1) * 512], psim[:],
                            pos_t[:, nk * 512:(nk + 1) * 512], op=ALU.add)
                    nc.vector.copy_predicated(sim[:], minv[:, t, :], cfill[:])
                    negmax = stp.tile([P, 1], FP, tag="negmax", name="negmax")
                    nc.vector.tensor_reduce(negmax[:], sim[:], axis=AX.X,
                                            op=ALU.max, negate=True)
                    pexp = ppool.tile([P, SEQ], BF, tag="pexp", name="pexp")
                    ssum = stp.tile([P, 1], FP, tag="ssum", name="ssum")
                    nc.scalar.activation(pexp[:], sim[:], AF.Exp,
                                         bias=negmax[:], accum_out=ssum[:])
                    rec = stp.tile([P, 1], FP, tag="rec", name="rec")
                    nc.vector.reciprocal(rec[:], ssum[:])
                    nc.vector.tensor_scalar_mul(pexp[:], pexp[:], rec[:])
                    pexps.append(pexp)
                return pexps

            def pt_attn(h, pexps):
                """Emit P^T transposes + attn@v + oT copy for head h."""
                PT = ptsp.tile([P, NS, NQT, P], BF, tag="PT", name="PT")
                for t in range(NQT):
                    for s0 in range(0, NS, 4):
                        ppt = ps_pt.tile([P, 4 * P], BF, tag="ppt", name="ppt")
                        for s4 in range(4):
                            nc.tensor.transpose(
                                ppt[:, s4 * P:(s4 + 1) * P],
                                pexps[t][:, (s0 + s4) * P:(s0 + s4 + 1) * P],
                                identb[:])
                        nc.scalar.copy(PT[:, s0:s0 + 4, t, :], ppt[:])
                po = ps_o.tile([P, MQ], FP, tag="po", name="po")
                for s in range(NS):
                    nc.tensor.matmul(po[:], lhsT=vsb[:, s, :],
                                     rhs=PT[:, s, :, :],
                                     start=(s == 0), stop=(s == NS - 1))
                nc.vector.tensor_copy(oT[:, h, :], po[:])

            prev = None
            for h in range(H):
                cur = sim_softmax(h)
                if prev is not None:
                    pt_attn(h - 1, prev)
                prev = cur
            pt_attn(H - 1, prev)

        # ---- phase 4: output projection (bf16) ---------------------------
        with tc.tile_pool(name="wof", bufs=3) as wofp, \
             tc.tile_pool(name="wob", bufs=3) as wobp, \
             tc.tile_pool(name="osb", bufs=2) as osbp, \
             tc.tile_pool(name="ps_out", bufs=NQT * (DIM // 512),
                          space="PSUM") as ps_out:
            pouts = []
            for t in range(NQT):
                for nk in range(DIM // 512):
                    pouts.append(ps_out.tile([P, 512], FP, tag="pout",
                                             name=f"pout{t}_{nk}"))
            for h in range(H):
                wo_f = wofp.tile([P, DIM], FP, tag="wof", name="wo_f")
                nc.sync.dma_start(out=wo_f[:], in_=wo_d[h * DH:(h + 1) * DH, :])
                wo_b = wobp.tile([P, DIM], BF, tag="wob", name="wo_b")
                nc.scalar.copy(wo_b[:], wo_f[:])
                for t in range(NQT):
                    for nk in range(DIM // 512):
                        nc.tensor.matmul(pouts[t * (DIM // 512) + nk][:],
                                         lhsT=oT[:, h, t * P:(t + 1) * P],
                                         rhs=wo_b[:, nk * 512:(nk + 1) * 512],
                                         start=(h == 0), stop=(h == H - 1))
            for t in range(NQT):
                osb = osbp.tile([P, DIM], FP, tag="osb", name="osb")
                for nk in range(DIM // 512):
                    nc.scalar.copy(osb[:, nk * 512:(nk + 1) * 512],
                                   pouts[t * (DIM // 512) + nk][:])
                nc.sync.dma_start(out=out_d[t * P:(t + 1) * P, :], in_=osb[:])

    nc.compile()
    return nc


_NC = None


def kernel(**inputs):
    global _NC, last_exec_time_ns
    x = np.asarray(inputs["x"], dtype=np.float32)[0]          # [SEQ, DIM]
    pos = np.asarray(inputs["pos_bias"], dtype=np.float32)    # [H, SEQ, SEQ]
    gamma = np.asarray(inputs["gamma"], dtype=np.float32)
    wq = np.ascontiguousarray(np.asarray(inputs["wq"], dtype=np.float32))
    wk = np.ascontiguousarray(np.asarray(inputs["wk"], dtype=np.float32))
    wv = np.ascontiguousarray(np.asarray(inputs["wv"], dtype=np.float32))
    wo = np.ascontiguousarray(np.asarray(inputs["wo"], dtype=np.float32))
    mask = np.asarray(inputs["mask"])

    if _NC is None:
        _NC = build()

    gamma_t = np.ascontiguousarray(gamma.reshape(CD, P).T)
    x = np.ascontiguousarray(x)
    in_maps = []
    for m in range(N_CORES):
        q0 = m * MQ
        in_maps.append({
            "x": x,
            "xq": np.ascontiguousarray(x[q0:q0 + MQ]),
            "pb": np.ascontiguousarray(pos[:, q0:q0 + MQ, :]).reshape(
                H * MQ, SEQ),
            "minv": np.ascontiguousarray(
                (~mask[q0:q0 + MQ, :]).astype(np.uint8)),
            "gamma_t": gamma_t,
            "wq": wq, "wk": wk, "wv": wv, "wo": wo,
        })
    trace = os.environ.get("KERNEL_TRACE") == "1"
    res = run_bass_kernel_spmd(_NC, in_maps, core_ids=list(range(N_CORES)),
                               trace=trace)
    last_exec_time_ns = res.exec_time_ns
    out = np.concatenate([res.results[m]["out"] for m in range(N_CORES)],
                         axis=0)[None, ...]
    return out.astype(np.float32)


# revision 8
# speedup vs baseline: 1.0037x; 1.0037x over previous
"""Trainium2 Bass kernel for MQA attention with RMSNorm + positional bias.

Reference computation (per core, seq-sharded over 8 cores):
  xn = rmsnorm(x) * gamma
  q = (xn @ wq) * scale   (16 heads x 128)     k = xn @ wk    v = xn @ wv
  sim = q @ k^T + pos_bias ; masked (non-causal entries := 1e-10)
  attn = softmax(sim); out = (attn @ v, concat heads) @ wo

Sharding: core m owns query rows [256*m, 256*m+256). K/V (shared MQA head)
are computed replicated on every core from the full x. Each core emits its
256 rows of the final output; the host concatenates. No collectives.

Precision: q/k projections and q@k^T run in true fp32 (the softmax here is
argmax-sharp: logits have std ~2000, so low-precision matmuls flip argmax
rows and blow up the error). v projection, attn@v and the output projection
run in bf16 - these only need ~1e-3 relative accuracy.

Attention inner loop is software-pipelined: the PE stream for head h's
sim matmuls is emitted before head h-1's P^T transposes + attn@v, so the
PE works on h-1's tail while DVE/ACT run h's softmax.
"""

import os

import numpy as np

import concourse.bass as bass
import concourse.mybir as mybir
import concourse.tile as tile
from concourse import bacc, masks
from concourse.bass_utils import run_bass_kernel_spmd

SEQ = 2048
DIM = 2048
H = 16
DH = 128
P = 128
N_CORES = 8
MQ = SEQ // N_CORES      # 256 query rows per core
NQT = MQ // P            # 2 query tiles per core
CD = DIM // P            # 16 contraction chunks
NS = SEQ // P            # 16 seq tiles
SPG = 2                  # seq tiles per k/v projection group
SG = NS // SPG           # 8 groups
SCALE = DH ** -0.5
EPS = 1e-5
MASKV = 1e-10

FP = mybir.dt.float32
BF = mybir.dt.bfloat16
U8 = mybir.dt.uint8
AF = mybir.ActivationFunctionType
ALU = mybir.AluOpType
AX = mybir.AxisListType

last_exec_time_ns = None


def _rms_scale_rows(nc, pool, xt, tag):
    """In-place x *= rsqrt(mean(x^2)+eps) for a [P, DIM] tile."""
    sq = pool.tile([P, DIM], FP, tag="sq_scratch", name="sq_scratch", bufs=1)
    ssq = pool.tile([P, 1], FP, tag=f"ssq{tag}", name=f"ssq{tag}")
    nc.scalar.activation(sq[:], xt[:], AF.Square, accum_out=ssq[:])
    nc.vector.tensor_scalar(ssq[:], ssq[:], 1.0 / DIM, EPS, ALU.mult, ALU.add)
    nc.scalar.sqrt(ssq[:], ssq[:])
    nc.vector.reciprocal(ssq[:], ssq[:])
    nc.vector.tensor_scalar_mul(xt[:], xt[:], ssq[:])


def build():
    nc = bacc.Bacc("TRN2", target_bir_lowering=False, debug=False,
                   num_devices=N_CORES)
    x_d = nc.dram_tensor("x", [SEQ, DIM], FP, kind="ExternalInput")
    xq_d = nc.dram_tensor("xq", [MQ, DIM], FP, kind="ExternalInput")
    pb_d = nc.dram_tensor("pb", [H * MQ, SEQ], FP, kind="ExternalInput")
    minv_d = nc.dram_tensor("minv", [MQ, SEQ], U8, kind="ExternalInput")
    g_d = nc.dram_tensor("gamma_t", [P, CD], FP, kind="ExternalInput")
    wq_d = nc.dram_tensor("wq", [DIM, H * DH], FP, kind="ExternalInput")
    wk_d = nc.dram_tensor("wk", [DIM, DH], FP, kind="ExternalInput")
    wv_d = nc.dram_tensor("wv", [DIM, DH], FP, kind="ExternalInput")
    wo_d = nc.dram_tensor("wo", [H * DH, DIM], FP, kind="ExternalInput")
    out_d = nc.dram_tensor("out", [MQ, DIM], FP, kind="ExternalOutput")

    with tile.TileContext(nc) as tc, \
         tc.tile_pool(name="singles", bufs=1) as singles:
        # ---- persistent tiles --------------------------------------------
        ident = singles.tile([P, P], FP, tag="ident", name="ident")
        masks.make_identity(nc, ident[:])
        identb = singles.tile([P, P], BF, tag="identb", name="identb")
        masks.make_identity(nc, identb[:])
        gam = singles.tile([P, CD], FP, tag="gam", name="gam")
        nc.sync.dma_start(out=gam[:], in_=g_d[:])
        minv = singles.tile([P, NQT, SEQ], U8, tag="minv", name="minv")
        for t in range(NQT):
            nc.sync.dma_start(out=minv[:, t, :], in_=minv_d[t * P:(t + 1) * P, :])
        cfill = singles.tile([P, SEQ], FP, tag="cfill", name="cfill")
        nc.gpsimd.memset(cfill[:], MASKV)

        qT = singles.tile([P, H, MQ], FP, tag="qT", name="qT")
        kT = singles.tile([P, SEQ], FP, tag="kT", name="kT")
        vsb = singles.tile([P, NS, DH], BF, tag="vsb", name="vsb")
        oT = singles.tile([P, H, MQ], BF, tag="oT", name="oT")

        with tc.tile_pool(name="xnTqp", bufs=1) as xnTqp:
            xnTq = xnTqp.tile([P, CD, MQ], FP, tag="xnTq", name="xnTq")

            # ---- phase 0: own-row xn^T -----------------------------------
            with tc.tile_pool(name="ph0", bufs=2) as ph0, \
                 tc.tile_pool(name="pstr0", bufs=2, space="PSUM") as pstr0:
                xnq = []
                for t in range(NQT):
                    xt = ph0.tile([P, DIM], FP, tag=f"xq{t}", name=f"xq{t}")
                    nc.sync.dma_start(out=xt[:], in_=xq_d[t * P:(t + 1) * P, :])
                    _rms_scale_rows(nc, ph0, xt, f"q{t}")
                    xnq.append(xt)
                for c in range(CD):
                    pt = pstr0.tile([P, MQ], FP, tag="trq", name="trq")
                    for t in range(NQT):
                        nc.tensor.transpose(pt[:, t * P:(t + 1) * P],
                                            xnq[t][:, c * P:(c + 1) * P],
                                            ident[:])
                    nc.vector.tensor_scalar_mul(xnTq[:, c, :], pt[:],
                                                gam[:, c:c + 1])

            # ---- phase 2: k/v projection over full seq -------------------
            with tc.tile_pool(name="xs", bufs=2) as xsp, \
                 tc.tile_pool(name="kvw", bufs=1) as kvwp, \
                 tc.tile_pool(name="pstr", bufs=2, space="PSUM") as pstr, \
                 tc.tile_pool(name="psk", bufs=2, space="PSUM") as psk, \
                 tc.tile_pool(name="psv", bufs=2, space="PSUM") as psv:
                wk_sb = kvwp.tile([P, CD, DH], FP, tag="wk", name="wk_sb")
                wv_sb = kvwp.tile([P, CD, DH], FP, tag="wv", name="wv_sb")
                wv_bf = kvwp.tile([P, CD, DH], BF, tag="wvb", name="wv_bf")
                for c in range(CD):
                    nc.sync.dma_start(out=wk_sb[:, c, :],
                                      in_=wk_d[c * P:(c + 1) * P, :])
                    nc.sync.dma_start(out=wv_sb[:, c, :],
                                      in_=wv_d[c * P:(c + 1) * P, :])
                    nc.scalar.copy(wv_bf[:, c, :], wv_sb[:, c, :])
                xnT = kvwp.tile([P, CD, SPG * P], FP, tag="xnT", name="xnT")
                xnTb = kvwp.tile([P, CD, SPG * P], BF, tag="xnTb", name="xnTb")
                vTs = kvwp.tile([P, SPG * P], FP, tag="vTs", name="vTs")
                for sg in range(SG):
                    xns = []
                    for s4 in range(SPG):
                        s = sg * SPG + s4
                        xt = xsp.tile([P, DIM], FP, tag=f"xs{s4}",
                                      name=f"xs{s4}")
                        nc.sync.dma_start(out=xt[:],
                                          in_=x_d[s * P:(s + 1) * P, :])
                        _rms_scale_rows(nc, xsp, xt, f"s{s4}")
                        xns.append(xt)
                    for c in range(CD):
                        pt = pstr.tile([P, SPG * P], FP, tag="trs", name="trs")
                        for s4 in range(SPG):
                            nc.tensor.transpose(pt[:, s4 * P:(s4 + 1) * P],
                                                xns[s4][:, c * P:(c + 1) * P],
                                                ident[:])
                        nc.vector.tensor_scalar_mul(xnT[:, c, :], pt[:],
                                                    gam[:, c:c + 1])
                        nc.scalar.copy(xnTb[:, c, :], xnT[:, c, :])
                    pk = psk.tile([P, SPG * P], FP, tag="pk", name="pk")
                    for c in range(CD):
                        nc.tensor.matmul(pk[:], lhsT=wk_sb[:, c, :],
                                         rhs=xnT[:, c, :],
                                         start=(c == 0), stop=(c == CD - 1))
                    nc.scalar.copy(kT[:, sg * SPG * P:(sg + 1) * SPG * P],
                                   pk[:])
                    pv = psv.tile([P, SPG * P], FP, tag="pv", name="pv")
                    for c in range(CD):
                        nc.tensor.matmul(pv[:], lhsT=wv_bf[:, c, :],
                                         rhs=xnTb[:, c, :],
                                         start=(c == 0), stop=(c == CD - 1))
                    nc.vector.tensor_copy(vTs[:], pv[:])
                    for s4 in range(SPG):
                        s = sg * SPG + s4
                        ptv = pstr.tile([P, P], FP, tag="vtr", name="vtr")
                        nc.tensor.transpose(ptv[:],
                                            vTs[:, s4 * P:(s4 + 1) * P],
                                            ident[:])
                        nc.vector.tensor_copy(vsb[:, s, :], ptv[:])

            # ---- phase 1: q projection (fp32) ----------------------------
            with tc.tile_pool(name="wqp", bufs=8) as wqp, \
                 tc.tile_pool(name="psq", bufs=2, space="PSUM") as psq:
                for h in range(H):
                    pq = psq.tile([P, MQ], FP, tag="pq", name="pq")
                    for c in range(CD):
                        wt = wqp.tile([P, P], FP, tag="wq", name="wqt")
                        nc.sync.dma_start(
                            out=wt[:],
                            in_=wq_d[c * P:(c + 1) * P, h * DH:(h + 1) * DH])
                        nc.tensor.matmul(pq[:], lhsT=wt[:], rhs=xnTq[:, c, :],
                                         start=(c == 0), stop=(c == CD - 1))
                    nc.scalar.mul(qT[:, h, :], pq[:], SCALE)

        # ---- phase 3: attention, software-pipelined over heads -----------
        with tc.tile_pool(name="pos", bufs=2) as posp, \
             tc.tile_pool(name="simp", bufs=2) as simp, \
             tc.tile_pool(name="pp", bufs=3) as ppool, \
             tc.tile_pool(name="pts", bufs=2) as ptsp, \
             tc.tile_pool(name="st", bufs=8) as stp, \
             tc.tile_pool(name="wof", bufs=2) as wofp, \
             tc.tile_pool(name="wob", bufs=12) as wobp, \
             tc.tile_pool(name="ps_sim", bufs=4, space="PSUM") as ps_sim, \
             tc.tile_pool(name="ps_pt", bufs=2, space="PSUM") as ps_pt, \
             tc.tile_pool(name="ps_o", bufs=2, space="PSUM") as ps_o:
            wo_tiles = []

            def wo_prefetch(h):
                wo_f = wofp.tile([P, DIM], FP, tag="wof", name="wo_f")
                nc.sync.dma_start(out=wo_f[:],
                                  in_=wo_d[h * DH:(h + 1) * DH, :])
                wo_b = wobp.tile([P, DIM], BF, tag="wob", name="wo_b")
                nc.scalar.copy(wo_b[:], wo_f[:])
                wo_tiles.append(wo_b)

            def sim_softmax(h):
                """Emit sim matmuls + softmax for head h; return pexp tiles."""
                pexps = []
                for t in range(NQT):
                    pos_t = posp.tile([P, SEQ], FP, tag="pos", name="pos")
                    nc.sync.dma_start(
                        out=pos_t[:],
                        in_=pb_d[h * MQ + t * P: h * MQ + (t + 1) * P, :])
                    sim = simp.tile([P, SEQ], FP, tag="sim", name="sim")
                    for nk in range(SEQ // 512):
                        psim = ps_sim.tile([P, 512], FP, tag="psim",
                                           name="psim")
                        nc.tensor.matmul(psim[:],
                                         lhsT=qT[:, h, t * P:(t + 1) * P],
                                         rhs=kT[:, nk * 512:(nk + 1) * 512],
                                         start=True, stop=True)
                        nc.scalar.copy(sim[:, nk * 512:(nk + 1) * 512],
                                       psim[:])
                        nc.gpsimd.tensor_tensor(
                            sim[:, nk * 512:(nk + 1) * 512],
                            sim[:, nk * 512:(nk + 1) * 512],
                            pos_t[:, nk * 512:(nk + 1) * 512], op=ALU.add)
                    nc.vector.copy_predicated(sim[:], minv[:, t, :], cfill[:])
                    negmax = stp.tile([P, 1], FP, tag="negmax", name="negmax")
                    nc.vector.tensor_reduce(negmax[:], sim[:], axis=AX.X,
                                            op=ALU.max, negate=True)
                    pexp = ppool.tile([P, SEQ], BF, tag="pexp", name="pexp")
                    ssum = stp.tile([P, 1], FP, tag="ssum", name="ssum")
                    nc.scalar.activation(pexp[:], sim[:], AF.Exp,
                                         bias=negmax[:], accum_out=ssum[:])
                    rec = stp.tile([P, 1], FP, tag="rec", name="rec")
                    nc.vector.reciprocal(rec[:], ssum[:])
                    nc.vector.tensor_scalar_mul(pexp[:], pexp[:], rec[:])
                    pexps.append(pexp)
                return pexps

            def pt_attn(h, pexps):
                """Emit P^T transposes + attn@v + oT copy for head h."""
                PT = ptsp.tile([P, NS, NQT, P], BF, tag="PT", name="PT")
                for t in range(NQT):
                    for s0 in range(0, NS, 4):
                        ppt = ps_pt.tile([P, 4 * P], BF, tag="ppt", name="ppt")
                        for s4 in range(4):
                            nc.tensor.transpose(
                                ppt[:, s4 * P:(s4 + 1) * P],
                                pexps[t][:, (s0 + s4) * P:(s0 + s4 + 1) * P],
                                identb[:])
                        nc.vector.tensor_copy(PT[:, s0:s0 + 4, t, :], ppt[:])
                po = ps_o.tile([P, MQ], FP, tag="po", name="po")
                for s in range(NS):
                    nc.tensor.matmul(po[:], lhsT=vsb[:, s, :],
                                     rhs=PT[:, s, :, :],
                                     start=(s == 0), stop=(s == NS - 1))
                nc.vector.tensor_copy(oT[:, h, :], po[:])

            prev = None
            for h in range(H):
                cur = sim_softmax(h)
                wo_prefetch(h)
                if prev is not None:
                    pt_attn(h - 1, prev)
                prev = cur
            pt_attn(H - 1, prev)

        # ---- phase 4: output projection (bf16) ---------------------------
        with tc.tile_pool(name="osb", bufs=2) as osbp, \
             tc.tile_pool(name="ps_out", bufs=NQT * (DIM // 512),
                          space="PSUM") as ps_out:
            pouts = []
            for t in range(NQT):
                for nk in range(DIM // 512):
                    pouts.append(ps_out.tile([P, 512], FP, tag="pout",
                                             name=f"pout{t}_{nk}"))
            for h in range(H):
                wo_b = wo_tiles[h]
                for t in range(NQT):
                    for nk in range(DIM // 512):
                        nc.tensor.matmul(pouts[t * (DIM // 512) + nk][:],
                                         lhsT=oT[:, h, t * P:(t + 1) * P],
                                         rhs=wo_b[:, nk * 512:(nk + 1) * 512],
                                         start=(h == 0), stop=(h == H - 1))
            for t in range(NQT):
                osb = osbp.tile([P, DIM], FP, tag="osb", name="osb")
                for nk in range(DIM // 512):
                    nc.scalar.copy(osb[:, nk * 512:(nk + 1) * 512],
                                   pouts[t * (DIM // 512) + nk][:])
                nc.sync.dma_start(out=out_d[t * P:(t + 1) * P, :], in_=osb[:])

    nc.compile()
    return nc


_NC = None


def kernel(**inputs):
    global _NC, last_exec_time_ns
    x = np.asarray(inputs["x"], dtype=np.float32)[0]          # [SEQ, DIM]
    pos = np.asarray(inputs["pos_bias"], dtype=np.float32)    # [H, SEQ, SEQ]
    gamma = np.asarray(inputs["gamma"], dtype=np.float32)
    wq = np.ascontiguousarray(np.asarray(inputs["wq"], dtype=np.float32))
    wk = np.ascontiguousarray(np.asarray(inputs["wk"], dtype=np.float32))
    wv = np.ascontiguousarray(np.asarray(inputs["wv"], dtype=np.float32))
    wo = np.ascontiguousarray(np.asarray(inputs["wo"], dtype=np.float32))
    mask = np.asarray(inputs["mask"])

    if _NC is None:
        _NC = build()

    gamma_t = np.ascontiguousarray(gamma.reshape(CD, P).T)
    x = np.ascontiguousarray(x)
    in_maps = []
    for m in range(N_CORES):
        q0 = m * MQ
        in_maps.append({
            "x": x,
            "xq": np.ascontiguousarray(x[q0:q0 + MQ]),
            "pb": np.ascontiguousarray(pos[:, q0:q0 + MQ, :]).reshape(
                H * MQ, SEQ),
            "minv": np.ascontiguousarray(
                (~mask[q0:q0 + MQ, :]).astype(np.uint8)),
            "gamma_t": gamma_t,
            "wq": wq, "wk": wk, "wv": wv, "wo": wo,
        })
    trace = os.environ.get("KERNEL_TRACE") == "1"
    res = run_bass_kernel_spmd(_NC, in_maps, core_ids=list(range(N_CORES)),
                               trace=trace)
    last_exec_time_ns = res.exec_time_ns
    out = np.concatenate([res.results[m]["out"] for m in range(N_CORES)],
                         axis=0)[None, ...]
    return out.astype(np.float32)
